# revision 17
# baseline (speedup 1.0000x reference)
"""CrossFrameAttention Trainium2 kernel.

Full (unsharded) inputs -> full output. Internally: data-parallel over the
fused frame*batch dim (F*B = 8 elements, one per NeuronCore), weights
replicated. Per core, a fused 1x1-conv QKV projection + softmax attention
written in Bass/Tile.

The wall-clock of a warm run is dominated by host-side costs (per-call jit
recompile + axon tunnel transfers), so the kernel is shaped around those:
  - the JAX persistent compilation cache is enabled so repeat runs skip the
    XLA+NEFF recompile entirely;
  - all device I/O is fp16 (x, weights in; attention output out), halving
    tunnel bytes;
  - the cheap epilogue (gamma * (attn + bv) + x residual) runs on the host
    in fp32, which also removes every PE transpose from the device kernel
    (output leaves the device in [N, C] layout).

Self-contained: hardcodes shapes from the problem spec.
"""

import numpy as np

import jax

# Warm runs create fresh jax.jit objects inside run_bass_kernel_spmd; the
# persistent cache turns their XLA+NEFF recompiles into disk hits.
jax.config.update("jax_compilation_cache_dir", "/tmp/jax_bass_cc")
jax.config.update("jax_persistent_cache_min_compile_time_secs", 0.0)
jax.config.update("jax_persistent_cache_min_entry_size_bytes", -1)

F, B, C, HH, WW = 4, 2, 256, 64, 64
N = HH * WW            # 4096 tokens per (frame,batch) element
FB = F * B             # 8 == n_cores
DQK = 32               # q/k channel dim (C/8)
NBLK = N // 512        # 8 query blocks of 512
NJ = N // 128          # 32 key chunks of 128

_CACHE = {}


def _build_nc():
    import concourse.mybir as mybir
    from concourse import bacc
    from concourse.tile import TileContext

    f32 = mybir.dt.float32
    f16 = mybir.dt.float16
    bf16 = mybir.dt.bfloat16
    AF = mybir.ActivationFunctionType
    ALU = mybir.AluOpType

    nc = bacc.Bacc(None, target_bir_lowering=False, debug=False)

    # single packed byte input per core: x as 12-bit fixed point (int8 "high"
    # plane + 4-bit residual nibbles, per-channel fp32 scale), weights/biases
    # as fp16 bytes. x = hi*s1 + lo*s2, s2 = s1/15; the nibble byte at col j
    # of block nb holds the low nibble for x col nb*512+j and the high nibble
    # for x col nb*512+256+j.
    u8 = mybir.dt.uint8
    i8 = mybir.dt.int8
    LHI, LLO = N, N // 2
    OW1 = LHI + LLO                  # wqk f16 bytes (64 cols -> 128 B)
    OW2 = OW1 + 2 * 2 * DQK          # wv f16 bytes (256 cols -> 512 B)
    OBQ = OW2 + 2 * C                # bqk f16 (2 B)
    OSC = OBQ + 4                    # x scale f32 (4 B), 4-byte aligned
    LPK = OSC + 4
    x_d = nc.dram_tensor("pack", [C, LPK], u8, kind="ExternalInput")
    wqk_d = x_d[:, OW1:OW2].bitcast(f16)
    wv_d = x_d[:, OW2:OW2 + 2 * C].bitcast(f16)
    bqk_d = x_d[:, OBQ:OBQ + 2].bitcast(f16)
    xsc_d = x_d[:, OSC:OSC + 4].bitcast(f32)
    # attention output, [token, channel] layout (transposed on host), int8
    # with one fp32 scale per token: attn = q * s. The softmax denominator is
    # folded into s, so the device never divides by it elementwise. The f32
    # scale is bit-packed into the last 4 int8 columns (single output tensor
    # -> single device->host fetch).
    i8 = mybir.dt.int8
    out_d = nc.dram_tensor("attnq", [N, C + 4], i8, kind="ExternalOutput")

    with TileContext(nc) as tc:
        with (
            tc.tile_pool(name="const", bufs=1) as cst,
            tc.tile_pool(name="xp", bufs=1) as xp,
            tc.tile_pool(name="xhp", bufs=4) as xhp,
            tc.tile_pool(name="xlp", bufs=4) as xlp,
            tc.tile_pool(name="qks", bufs=1) as qks,
            tc.tile_pool(name="vtp", bufs=1) as vtp,
            tc.tile_pool(name="ep", bufs=16) as ep,
            tc.tile_pool(name="aop", bufs=8) as aop,
            tc.tile_pool(name="rcp", bufs=8) as rcp,
            tc.tile_pool(name="ps_s", bufs=3, space="PSUM") as ps_s,
            tc.tile_pool(name="ps_av", bufs=4, space="PSUM") as ps_av,
        ):
            # ---- weights / biases ----
            wqk_t = [cst.tile([128, 2 * DQK], f16, tag=f"wqk{c}", name=f"wqk{c}")
                     for c in range(2)]
            wv_t = [cst.tile([128, C], f16, tag=f"wv{c}", name=f"wv{c}")
                    for c in range(2)]
            bq_t = cst.tile([DQK, 1], f32, tag="bq", name="bq")
            bk_t = cst.tile([DQK, 1], f32, tag="bk", name="bk")
            bq16 = cst.tile([DQK, 1], f16, tag="bq16", name="bq16")
            bk16 = cst.tile([DQK, 1], f16, tag="bk16", name="bk16")
            for c in range(2):
                nc.sync.dma_start(out=wqk_t[c], in_=wqk_d[c * 128:(c + 1) * 128, :])
                nc.sync.dma_start(out=wv_t[c], in_=wv_d[c * 128:(c + 1) * 128, :])
            nc.sync.dma_start(out=bq16, in_=bqk_d[0:DQK, :])
            nc.sync.dma_start(out=bk16, in_=bqk_d[DQK:2 * DQK, :])
            nc.scalar.activation(bq_t, bq16, AF.Copy)
            nc.scalar.activation(bk_t, bk16, AF.Copy)
            ones_bt = cst.tile([128, 1], bf16, tag="ones", name="ones_bt")
            nc.gpsimd.memset(ones_bt, 1.0)
            mask_t = cst.tile([128, 1], u8, tag="msk", name="mask_t")
            nc.gpsimd.memset(mask_t, 15)
            four_t = cst.tile([128, 1], u8, tag="fr", name="four_t")
            nc.gpsimd.memset(four_t, 4)
            s1_t, s2_t = [], []
            for c in range(2):
                s1 = cst.tile([128, 1], f32, tag=f"s1_{c}", name=f"s1_{c}")
                nc.sync.dma_start(out=s1, in_=xsc_d[c * 128:(c + 1) * 128, :])
                s2 = cst.tile([128, 1], f32, tag=f"s2_{c}", name=f"s2_{c}")
                nc.vector.tensor_scalar(s2, s1, 1.0 / 15.0, None, ALU.mult)
                s1_t.append(s1)
                s2_t.append(s2)

            # ---- x: 2 c-chunks x 8 n-blocks of [128, 512] f16, unpacked from
            # the 12-bit planes on VectorE ----
            x_t = [[xp.tile([128, 512], f16, tag=f"x{c}_{nb}", name=f"x{c}_{nb}")
                    for nb in range(NBLK)] for c in range(2)]
            for nb in range(NBLK):
                for c in range(2):
                    r0, r1 = c * 128, (c + 1) * 128
                    hi8 = xhp.tile([128, 512], i8, tag="hi", name=f"hi{c}_{nb}")
                    nc.sync.dma_start(
                        out=hi8,
                        in_=x_d[r0:r1, nb * 512:(nb + 1) * 512].bitcast(i8))
                    lo8 = xlp.tile([128, 256], u8, tag="lo", name=f"lo{c}_{nb}")
                    nc.sync.dma_start(
                        out=lo8,
                        in_=x_d[r0:r1, LHI + nb * 256:LHI + (nb + 1) * 256])
                    xt = x_t[c][nb]
                    nc.vector.tensor_scalar(xt, hi8, s1_t[c], None, ALU.mult)
                    le = xlp.tile([128, 256], u8, tag="lq", name=f"le{c}_{nb}")
                    nc.vector.tensor_scalar(le, lo8, mask_t, None,
                                            ALU.bitwise_and)
                    lh = xlp.tile([128, 256], u8, tag="lq2", name=f"lh{c}_{nb}")
                    nc.vector.tensor_scalar(lh, lo8, four_t, None,
                                            ALU.logical_shift_right)
                    te = xlp.tile([128, 256], f16, tag="tf", name=f"te{c}_{nb}")
                    nc.vector.tensor_scalar(te, le, s2_t[c], None, ALU.mult)
                    th = xlp.tile([128, 256], f16, tag="tf2", name=f"th{c}_{nb}")
                    nc.vector.tensor_scalar(th, lh, s2_t[c], None, ALU.mult)
                    nc.vector.tensor_add(xt[:, 0:256], xt[:, 0:256], te)
                    nc.vector.tensor_add(xt[:, 256:512], xt[:, 256:512], th)

            q_sb = qks.tile([DQK, N], f16, tag="q", name="q_sb")
            k_sb = qks.tile([DQK, N], f16, tag="k", name="k_sb")

            # ---- QK projection: q = Wq @ x, k = Wk @ x  (K=C contraction) ----
            for nb in range(NBLK):
                q_ps = ps_s.tile([DQK, 512], f32, tag="s", name=f"qps{nb}")
                nc.tensor.matmul(q_ps, lhsT=wqk_t[0][:, 0:DQK],
                                 rhs=x_t[0][nb], start=True, stop=False)
                nc.tensor.matmul(q_ps, lhsT=wqk_t[1][:, 0:DQK],
                                 rhs=x_t[1][nb], start=False, stop=True)
                nc.scalar.activation(q_sb[:, nb * 512:(nb + 1) * 512], q_ps,
                                     AF.Identity, bias=bq_t)
                k_ps = ps_s.tile([DQK, 512], f32, tag="s", name=f"kps{nb}")
                nc.tensor.matmul(k_ps, lhsT=wqk_t[0][:, DQK:2 * DQK],
                                 rhs=x_t[0][nb], start=True, stop=False)
                nc.tensor.matmul(k_ps, lhsT=wqk_t[1][:, DQK:2 * DQK],
                                 rhs=x_t[1][nb], start=False, stop=True)
                nc.scalar.activation(k_sb[:, nb * 512:(nb + 1) * 512], k_ps,
                                     AF.Identity, bias=bk_t)

            # ---- V projection, directly transposed: vT[j, c] = x[:, j].T @ WvT
            # vT tiles [128 (j), 257]; col 256 = 1.0 so the AV matmul also
            # produces sum_j(E) ("ones trick") for the softmax denominator.
            vt_t = []
            for j in range(NJ):
                nb, off = divmod(j * 128, 512)
                pv = ps_av.tile([128, C], f32, tag="av", name=f"vps{j}")
                nc.tensor.matmul(pv, lhsT=x_t[0][nb][:, off:off + 128],
                                 rhs=wv_t[0], start=True, stop=False)
                nc.tensor.matmul(pv, lhsT=x_t[1][nb][:, off:off + 128],
                                 rhs=wv_t[1], start=False, stop=True)
                vt = vtp.tile([128, C + 1], bf16, tag=f"vt{j}", name=f"vt{j}")
                nc.scalar.activation(vt[:, 0:C], pv, AF.Copy)
                nc.scalar.activation(vt[:, C:C + 1], ones_bt, AF.Copy)
                vt_t.append(vt)

            # ---- attention over 8 query blocks of 512 ----
            for ib in range(NBLK):
                av_ps = [ps_av.tile([128, C + 1], f32, tag="av", name=f"av{ib}_{q}")
                         for q in range(4)]
                e_t = {}
                for j in range(NJ):
                    s_ps = ps_s.tile([128, 512], f32, tag="s", name=f"sps{ib}_{j}")
                    nc.tensor.matmul(
                        s_ps, lhsT=k_sb[:, j * 128:(j + 1) * 128],
                        rhs=q_sb[:, ib * 512:(ib + 1) * 512],
                        start=True, stop=True)
                    et = ep.tile([128, 512], bf16, tag="e", name=f"e{ib}_{j}")
                    nc.scalar.activation(et, s_ps, AF.Exp)
                    e_t[j] = et
                    if j >= 2:
                        jj = j - 2
                        for q in range(4):
                            nc.tensor.matmul(
                                av_ps[q], lhsT=e_t[jj][:, q * 128:(q + 1) * 128],
                                rhs=vt_t[jj], start=(jj == 0), stop=False)
                for jj in (NJ - 2, NJ - 1):
                    for q in range(4):
                        nc.tensor.matmul(
                            av_ps[q], lhsT=e_t[jj][:, q * 128:(q + 1) * 128],
                            rhs=vt_t[jj], start=False, stop=(jj == NJ - 1))

                # int8 quantization: q = av * (127/maxabs(av)); the softmax
                # 1/sumexp and the maxabs/127 dequant step both land in the
                # per-token scale s = maxabs * (1/sumexp) / 127.
                for q in range(4):
                    mx = rcp.tile([128, 1], f32, tag="mx", name=f"mx{ib}_{q}")
                    nc.vector.tensor_reduce(
                        mx, av_ps[q][:, 0:C], axis=mybir.AxisListType.X,
                        op=ALU.max, apply_absolute_value=True)
                    mxc = rcp.tile([128, 1], f32, tag="mxc", name=f"mxc{ib}_{q}")
                    nc.vector.tensor_scalar(mxc, mx, 1e-30, None, ALU.max)
                    rq = rcp.tile([128, 1], f32, tag="rq", name=f"rq{ib}_{q}")
                    nc.vector.reciprocal(rq, mxc)
                    ao = aop.tile([128, C], i8, tag="ao", name=f"ao{ib}_{q}")
                    nc.vector.tensor_scalar(ao, av_ps[q][:, 0:C], rq, 127.0,
                                            ALU.mult, ALU.mult)
                    r0 = ib * 512 + q * 128
                    nc.sync.dma_start(out=out_d[r0:r0 + 128, 0:C], in_=ao)
                    rs = rcp.tile([128, 1], f32, tag="rs", name=f"rs{ib}_{q}")
                    nc.vector.reciprocal(rs, av_ps[q][:, C:C + 1])
                    st = rcp.tile([128, 1], f32, tag="st", name=f"st{ib}_{q}")
                    nc.vector.tensor_scalar(st, mxc, rs, 1.0 / 127.0,
                                            ALU.mult, ALU.mult)
                    nc.sync.dma_start(out=out_d[r0:r0 + 128, C:C + 4],
                                      in_=st.bitcast(i8))

    nc.finalize()
    return nc


def _run(in_maps, trace=False):
    from concourse.bass_utils import run_bass_kernel_spmd

    if "nc" not in _CACHE:
        _CACHE["nc"] = _build_nc()
    return run_bass_kernel_spmd(
        _CACHE["nc"], in_maps, list(range(FB)),
        trace=trace, trace_cores=[0] if trace else None)


def _prep_inputs(features, Wq, bq, Wk, bk, Wv, bv, gamma):
    LHI = N
    OW1 = LHI + N // 2
    OW2 = OW1 + 4 * DQK
    OBQ = OW2 + 2 * C
    OSC = OBQ + 4
    LPK = OSC + 4

    x_all = np.asarray(features, dtype=np.float32).reshape(FB, C, N)
    # 12-bit fixed point: x ~= hi*s1 + lo*s2, hi int8, lo in [0,15], s2=s1/15
    mx = np.abs(x_all).max(axis=2, keepdims=True)
    s1 = np.maximum(mx, 1e-30) * (1.0 / 127.0)               # [FB, C, 1]
    s2 = s1 * (1.0 / 15.0)
    h = np.round(x_all / s1 - 0.5)
    np.clip(h, -128, 127, out=h)
    l = np.round((x_all - h * s1) / s2)
    np.clip(l, 0, 15, out=l)
    hi8 = h.astype(np.int8)
    l5 = l.astype(np.uint8).reshape(FB, C, NBLK, 2, 256)
    lo = (l5[:, :, :, 0, :] | (l5[:, :, :, 1, :] << 4)).reshape(FB, C, N // 2)

    wqkT = np.concatenate([np.asarray(Wq), np.asarray(Wk)], axis=0).T  # [C, 64]
    wvT = np.asarray(Wv).T                                             # [C, C]
    bqk_col = np.zeros((C, 1), np.float32)
    bqk_col[0:DQK, 0] = np.asarray(bq, dtype=np.float32)
    bqk_col[DQK:2 * DQK, 0] = np.asarray(bk, dtype=np.float32)

    pack = np.empty((FB, C, LPK), np.uint8)
    pack[:, :, 0:N] = hi8.view(np.uint8)
    pack[:, :, N:OW1] = lo
    pack[:, :, OW1:OW2] = np.ascontiguousarray(
        wqkT.astype(np.float16)).view(np.uint8)
    pack[:, :, OW2:OW2 + 2 * C] = np.ascontiguousarray(
        wvT.astype(np.float16)).view(np.uint8)
    pack[:, :, OBQ:OBQ + 2] = bqk_col.astype(np.float16).view(np.uint8)
    pack[:, :, OBQ + 2:OSC] = 0
    pack[:, :, OSC:OSC + 4] = s1.astype(np.float32).view(np.uint8)
    return [{"pack": pack[i]} for i in range(FB)]


def kernel(features, Wq, bq, Wk, bk, Wv, bv, gamma):
    in_maps = _prep_inputs(features, Wq, bq, Wk, bk, Wv, bv, gamma)
    res = _run(in_maps, trace=False)
    # device returns int8 attention with per-token fp32 scales bit-packed in
    # the last 4 columns; dequant + the epilogue gamma * (attn + bv) + x run
    # here in fp32.
    raw = np.stack([res.results[i]["attnq"] for i in range(FB)], axis=0)
    scales = np.ascontiguousarray(raw[:, :, C:C + 4]).view(np.float32)
    attn = raw[:, :, 0:C].astype(np.float32) * scales        # [FB, N, C]
    attn = attn.transpose(0, 2, 1)                           # [FB, C, N]
    x_all = np.asarray(features, dtype=np.float32).reshape(FB, C, N)
    g = np.float32(np.asarray(gamma, dtype=np.float32).reshape(-1)[0])
    bvv = np.asarray(bv, dtype=np.float32).reshape(1, C, 1)
    out = g * (attn + bvv) + x_all
    return out.reshape(F, B, C, HH, WW).astype(np.float32)


# revision 18
# speedup vs baseline: 1.2665x; 1.2665x over previous
"""CrossFrameAttention Trainium2 kernel.

Full (unsharded) inputs -> full output. Internally: data-parallel over the
fused frame*batch dim (F*B = 8 elements, one per NeuronCore), weights
replicated. Per core, a fused 1x1-conv QKV projection + softmax attention
written in Bass/Tile.

The wall-clock of a warm run is dominated by host-side costs (per-call jit
recompile + axon tunnel transfers), so the kernel is shaped around those:
  - the JAX persistent compilation cache is enabled so repeat runs skip the
    XLA+NEFF recompile entirely;
  - all device I/O is fp16 (x, weights in; attention output out), halving
    tunnel bytes;
  - the cheap epilogue (gamma * (attn + bv) + x residual) runs on the host
    in fp32, which also removes every PE transpose from the device kernel
    (output leaves the device in [N, C] layout).

Self-contained: hardcodes shapes from the problem spec.
"""

import numpy as np

import jax

# Warm runs create fresh jax.jit objects inside run_bass_kernel_spmd; the
# persistent cache turns their XLA+NEFF recompiles into disk hits.
jax.config.update("jax_compilation_cache_dir", "/tmp/jax_bass_cc")
jax.config.update("jax_persistent_cache_min_compile_time_secs", 0.0)
jax.config.update("jax_persistent_cache_min_entry_size_bytes", -1)

F, B, C, HH, WW = 4, 2, 256, 64, 64
N = HH * WW            # 4096 tokens per (frame,batch) element
FB = F * B             # 8 == n_cores
DQK = 32               # q/k channel dim (C/8)
NBLK = N // 512        # 8 query blocks of 512
NJ = N // 128          # 32 key chunks of 128

_CACHE = {}


def _build_nc():
    import concourse.mybir as mybir
    from concourse import bacc
    from concourse.tile import TileContext

    f32 = mybir.dt.float32
    f16 = mybir.dt.float16
    bf16 = mybir.dt.bfloat16
    AF = mybir.ActivationFunctionType
    ALU = mybir.AluOpType

    nc = bacc.Bacc(None, target_bir_lowering=False, debug=False)

    # single packed fp16 input: [x | WqT Wk T | WvT | bqk] along columns —
    # one host->device tensor per call instead of four (fewer per-array RPCs)
    PACK = N + 2 * DQK + C + 1
    x_d = nc.dram_tensor("pack", [C, PACK], f16, kind="ExternalInput")
    wqk_d = x_d[:, N:N + 2 * DQK]
    wv_d = x_d[:, N + 2 * DQK:N + 2 * DQK + C]
    bqk_d = x_d[:, PACK - 1:PACK]
    # attention output, [token, channel] layout (transposed on host), int8
    # with one fp32 scale per token: attn = q * s. The softmax denominator is
    # folded into s, so the device never divides by it elementwise. The f32
    # scale is bit-packed into the last 4 int8 columns (single output tensor
    # -> single device->host fetch).
    i8 = mybir.dt.int8
    out_d = nc.dram_tensor("attnq", [N, C + 4], i8, kind="ExternalOutput")

    with TileContext(nc) as tc:
        with (
            tc.tile_pool(name="const", bufs=1) as cst,
            tc.tile_pool(name="xp", bufs=1) as xp,
            tc.tile_pool(name="qks", bufs=1) as qks,
            tc.tile_pool(name="vtp", bufs=1) as vtp,
            tc.tile_pool(name="ep", bufs=16) as ep,
            tc.tile_pool(name="aop", bufs=8) as aop,
            tc.tile_pool(name="rcp", bufs=8) as rcp,
            tc.tile_pool(name="ps_s", bufs=3, space="PSUM") as ps_s,
            tc.tile_pool(name="ps_av", bufs=4, space="PSUM") as ps_av,
        ):
            # ---- weights / biases ----
            wqk_t = [cst.tile([128, 2 * DQK], f16, tag=f"wqk{c}", name=f"wqk{c}")
                     for c in range(2)]
            wv_t = [cst.tile([128, C], f16, tag=f"wv{c}", name=f"wv{c}")
                    for c in range(2)]
            bq_t = cst.tile([DQK, 1], f32, tag="bq", name="bq")
            bk_t = cst.tile([DQK, 1], f32, tag="bk", name="bk")
            bq16 = cst.tile([DQK, 1], f16, tag="bq16", name="bq16")
            bk16 = cst.tile([DQK, 1], f16, tag="bk16", name="bk16")
            for c in range(2):
                nc.sync.dma_start(out=wqk_t[c], in_=wqk_d[c * 128:(c + 1) * 128, :])
                nc.sync.dma_start(out=wv_t[c], in_=wv_d[c * 128:(c + 1) * 128, :])
            nc.sync.dma_start(out=bq16, in_=bqk_d[0:DQK, :])
            nc.sync.dma_start(out=bk16, in_=bqk_d[DQK:2 * DQK, :])
            nc.scalar.activation(bq_t, bq16, AF.Copy)
            nc.scalar.activation(bk_t, bk16, AF.Copy)
            ones_bt = cst.tile([128, 1], bf16, tag="ones", name="ones_bt")
            nc.gpsimd.memset(ones_bt, 1.0)

            # ---- x: 2 c-chunks x 8 n-blocks of [128, 512], fp16 ----
            x_t = [[xp.tile([128, 512], f16, tag=f"x{c}_{nb}", name=f"x{c}_{nb}")
                    for nb in range(NBLK)] for c in range(2)]
            for nb in range(NBLK):
                for c in range(2):
                    nc.sync.dma_start(
                        out=x_t[c][nb],
                        in_=x_d[c * 128:(c + 1) * 128, nb * 512:(nb + 1) * 512])

            q_sb = qks.tile([DQK, N], f16, tag="q", name="q_sb")
            k_sb = qks.tile([DQK, N], f16, tag="k", name="k_sb")

            # ---- QK projection: q = Wq @ x, k = Wk @ x  (K=C contraction) ----
            for nb in range(NBLK):
                q_ps = ps_s.tile([DQK, 512], f32, tag="s", name=f"qps{nb}")
                nc.tensor.matmul(q_ps, lhsT=wqk_t[0][:, 0:DQK],
                                 rhs=x_t[0][nb], start=True, stop=False)
                nc.tensor.matmul(q_ps, lhsT=wqk_t[1][:, 0:DQK],
                                 rhs=x_t[1][nb], start=False, stop=True)
                nc.scalar.activation(q_sb[:, nb * 512:(nb + 1) * 512], q_ps,
                                     AF.Identity, bias=bq_t)
                k_ps = ps_s.tile([DQK, 512], f32, tag="s", name=f"kps{nb}")
                nc.tensor.matmul(k_ps, lhsT=wqk_t[0][:, DQK:2 * DQK],
                                 rhs=x_t[0][nb], start=True, stop=False)
                nc.tensor.matmul(k_ps, lhsT=wqk_t[1][:, DQK:2 * DQK],
                                 rhs=x_t[1][nb], start=False, stop=True)
                nc.scalar.activation(k_sb[:, nb * 512:(nb + 1) * 512], k_ps,
                                     AF.Identity, bias=bk_t)

            # ---- V projection, directly transposed: vT[j, c] = x[:, j].T @ WvT
            # vT tiles [128 (j), 257]; col 256 = 1.0 so the AV matmul also
            # produces sum_j(E) ("ones trick") for the softmax denominator.
            vt_t = []
            for j in range(NJ):
                nb, off = divmod(j * 128, 512)
                pv = ps_av.tile([128, C], f32, tag="av", name=f"vps{j}")
                nc.tensor.matmul(pv, lhsT=x_t[0][nb][:, off:off + 128],
                                 rhs=wv_t[0], start=True, stop=False)
                nc.tensor.matmul(pv, lhsT=x_t[1][nb][:, off:off + 128],
                                 rhs=wv_t[1], start=False, stop=True)
                vt = vtp.tile([128, C + 1], bf16, tag=f"vt{j}", name=f"vt{j}")
                nc.scalar.activation(vt[:, 0:C], pv, AF.Copy)
                nc.scalar.activation(vt[:, C:C + 1], ones_bt, AF.Copy)
                vt_t.append(vt)

            # ---- attention over 8 query blocks of 512 ----
            for ib in range(NBLK):
                av_ps = [ps_av.tile([128, C + 1], f32, tag="av", name=f"av{ib}_{q}")
                         for q in range(4)]
                e_t = {}
                for j in range(NJ):
                    s_ps = ps_s.tile([128, 512], f32, tag="s", name=f"sps{ib}_{j}")
                    nc.tensor.matmul(
                        s_ps, lhsT=k_sb[:, j * 128:(j + 1) * 128],
                        rhs=q_sb[:, ib * 512:(ib + 1) * 512],
                        start=True, stop=True)
                    et = ep.tile([128, 512], bf16, tag="e", name=f"e{ib}_{j}")
                    nc.scalar.activation(et, s_ps, AF.Exp)
                    e_t[j] = et
                    if j >= 2:
                        jj = j - 2
                        for q in range(4):
                            nc.tensor.matmul(
                                av_ps[q], lhsT=e_t[jj][:, q * 128:(q + 1) * 128],
                                rhs=vt_t[jj], start=(jj == 0), stop=False)
                for jj in (NJ - 2, NJ - 1):
                    for q in range(4):
                        nc.tensor.matmul(
                            av_ps[q], lhsT=e_t[jj][:, q * 128:(q + 1) * 128],
                            rhs=vt_t[jj], start=False, stop=(jj == NJ - 1))

                # int8 quantization: q = av * (127/maxabs(av)); the softmax
                # 1/sumexp and the maxabs/127 dequant step both land in the
                # per-token scale s = maxabs * (1/sumexp) / 127.
                for q in range(4):
                    mx = rcp.tile([128, 1], f32, tag="mx", name=f"mx{ib}_{q}")
                    nc.vector.tensor_reduce(
                        mx, av_ps[q][:, 0:C], axis=mybir.AxisListType.X,
                        op=ALU.max, apply_absolute_value=True)
                    mxc = rcp.tile([128, 1], f32, tag="mxc", name=f"mxc{ib}_{q}")
                    nc.vector.tensor_scalar(mxc, mx, 1e-30, None, ALU.max)
                    rq = rcp.tile([128, 1], f32, tag="rq", name=f"rq{ib}_{q}")
                    nc.vector.reciprocal(rq, mxc)
                    ao = aop.tile([128, C], i8, tag="ao", name=f"ao{ib}_{q}")
                    nc.vector.tensor_scalar(ao, av_ps[q][:, 0:C], rq, 127.0,
                                            ALU.mult, ALU.mult)
                    r0 = ib * 512 + q * 128
                    nc.sync.dma_start(out=out_d[r0:r0 + 128, 0:C], in_=ao)
                    rs = rcp.tile([128, 1], f32, tag="rs", name=f"rs{ib}_{q}")
                    nc.vector.reciprocal(rs, av_ps[q][:, C:C + 1])
                    st = rcp.tile([128, 1], f32, tag="st", name=f"st{ib}_{q}")
                    nc.vector.tensor_scalar(st, mxc, rs, 1.0 / 127.0,
                                            ALU.mult, ALU.mult)
                    nc.sync.dma_start(out=out_d[r0:r0 + 128, C:C + 4],
                                      in_=st.bitcast(i8))

    nc.finalize()
    return nc


def _run(in_maps, trace=False):
    from concourse.bass_utils import run_bass_kernel_spmd

    if "nc" not in _CACHE:
        _CACHE["nc"] = _build_nc()
    return run_bass_kernel_spmd(
        _CACHE["nc"], in_maps, list(range(FB)),
        trace=trace, trace_cores=[0] if trace else None)


def _prep_inputs(features, Wq, bq, Wk, bk, Wv, bv, gamma):
    x_all = np.asarray(features, dtype=np.float32).reshape(FB, C, N)
    wqkT = np.concatenate([np.asarray(Wq), np.asarray(Wk)], axis=0).T  # [C, 64]
    wvT = np.asarray(Wv).T                                             # [C, C]
    bqk_col = np.zeros((C, 1), np.float32)
    bqk_col[0:DQK, 0] = np.asarray(bq, dtype=np.float32)
    bqk_col[DQK:2 * DQK, 0] = np.asarray(bk, dtype=np.float32)
    pack = np.empty((FB, C, N + 2 * DQK + C + 1), np.float16)
    pack[:, :, 0:N] = x_all
    pack[:, :, N:N + 2 * DQK] = wqkT.astype(np.float16)
    pack[:, :, N + 2 * DQK:N + 2 * DQK + C] = wvT.astype(np.float16)
    pack[:, :, N + 2 * DQK + C:] = bqk_col.astype(np.float16)
    return [{"pack": pack[i]} for i in range(FB)]


def kernel(features, Wq, bq, Wk, bk, Wv, bv, gamma):
    in_maps = _prep_inputs(features, Wq, bq, Wk, bk, Wv, bv, gamma)
    res = _run(in_maps, trace=False)
    # device returns int8 attention with per-token fp32 scales bit-packed in
    # the last 4 columns; dequant + the epilogue gamma * (attn + bv) + x run
    # here in fp32.
    raw = np.stack([res.results[i]["attnq"] for i in range(FB)], axis=0)
    scales = np.ascontiguousarray(raw[:, :, C:C + 4]).view(np.float32)
    attn = raw[:, :, 0:C].astype(np.float32) * scales        # [FB, N, C]
    attn = attn.transpose(0, 2, 1)                           # [FB, C, N]
    x_all = np.asarray(features, dtype=np.float32).reshape(FB, C, N)
    g = np.float32(np.asarray(gamma, dtype=np.float32).reshape(-1)[0])
    bvv = np.asarray(bv, dtype=np.float32).reshape(1, C, 1)
    out = g * (attn + bvv) + x_all
    return out.reshape(F, B, C, HH, WW).astype(np.float32)
